# revision 1
# baseline (speedup 1.0000x reference)
"""Trainium2 Bass kernel for the quantum ConvLSTM reference.

Math reduction (validated to ~3e-7 vs the jax reference):
  * quantum_conv(patch) == T16[b] where b is the 4-bit index from
    thresholding the 4 pixels at 127 (the RX encoder maps each patch to a
    computational basis state; the fixed RY/CNOT circuit then gives a
    fixed 16-entry probability table). T16 is evaluated on-chip as a
    multilinear polynomial in the 4 bits.
  * qlayer(x, p) == [z1*z2*z3, z0*z1, z0*z1*z2, z0*z1*z2*z3] with
    z_w = cos(x_w + p_w) (RX angles add; the CNOT ring XORs independent
    wire bits, so <Z> factorizes into products of cos).
  * The LSTM scan then becomes a tiny classical recurrence evaluated with
    DVE/ACT ops: theta = Wx*conv + Wh@h + b + rx; z = cos(theta);
    gate products; sigmoid via tanh(q/2) (keeps Sin+Tanh in ONE ACT
    table set); c/h updates with 2c/2h scaling folded into constants.

Sharding: pure data parallel over batch (2048 -> 8 cores x 256 rows).
Each core: 128 partitions x 2 column-groups; seq scan of 512 steps fully
unrolled on-chip.
"""

import sys

import numpy as np

sys.path.insert(0, "/opt/trn_rl_repo")

N_CORES = 8
SIDE_ON_GPSIMD = False
BATCH = 2048
SEQ = 512
BPC = BATCH // N_CORES          # 256 batch rows per core
GRP = BPC // 128                # 2 column groups of 128 partitions
PI = float(np.pi)

# ---------------------------------------------------------------------------
# Host-side constants (input independent): T16 lookup table + multilinear
# coefficients. _RY_ANGLES is the fixed seed-0 table from the reference.
# ---------------------------------------------------------------------------
_RY_ANGLES = np.random.RandomState(0).uniform(0.0, 2.0 * np.pi, size=(2, 4)).astype(np.float32)


def _build_t16() -> np.ndarray:
    s = np.zeros((16, 2, 2, 2, 2), np.complex64)
    for b in range(16):
        bits = [(b >> 3) & 1, (b >> 2) & 1, (b >> 1) & 1, b & 1]
        s[(b, *bits)] = 1.0

    def ry(state, th, w):
        a0 = np.take(state, 0, axis=1 + w)
        a1 = np.take(state, 1, axis=1 + w)
        c = np.complex64(np.cos(np.float32(th) / 2))
        sn = np.complex64(np.sin(np.float32(th) / 2))
        return np.stack([c * a0 - sn * a1, sn * a0 + c * a1], axis=1 + w)

    def cnot(state, ctl, tgt):
        s0 = np.take(state, 0, axis=1 + ctl)
        s1 = np.take(state, 1, axis=1 + ctl)
        t_ax = 1 + tgt if tgt < ctl else tgt
        s1 = np.flip(s1, axis=t_ax)
        return np.stack([s0, s1], axis=1 + ctl)

    for layer in range(2):
        for w in range(4):
            s = ry(s, _RY_ANGLES[layer, w], w)
        for w in range(3):
            s = cnot(s, w, w + 1)
    probs = np.abs(s) ** 2
    cols = []
    for w in range(4):
        other = tuple(a for a in range(1, 5) if a != 1 + w)
        cols.append(probs.sum(axis=other)[:, 1])
    return np.stack(cols, axis=1).mean(axis=1)  # (16,)


def _multilinear_coeffs(t16: np.ndarray) -> np.ndarray:
    """C[4][4] with T16[b] = sum_jk C[j,k]*u_j*v_k, u=[1,b0,b1,b0b1], v=[1,b2,b3,b2b3]."""
    m = np.zeros((16, 16))
    for b in range(16):
        b0, b1, b2, b3 = (b >> 3) & 1, (b >> 2) & 1, (b >> 1) & 1, b & 1
        u = [1, b0, b1, b0 * b1]
        v = [1, b2, b3, b2 * b3]
        for j in range(4):
            for k in range(4):
                m[b, j * 4 + k] = u[j] * v[k]
    return np.linalg.solve(m, t16.astype(np.float64)).reshape(4, 4)


_T16 = _build_t16()
_CML = _multilinear_coeffs(_T16)


def _fit_odd(f, hi, ncoef):
    """Near-minimax odd fit f(x) ~ x*p(x^2) on [-hi, hi]; returns p coeffs
    c[0..ncoef-1] (ascending powers of x^2)."""
    n = 4000
    k = np.arange(n)
    x = np.cos(np.pi * (k + 0.5) / n) * hi
    y = x * x
    a = np.stack([x * y ** j for j in range(ncoef)], axis=1)
    tgt = f(x)
    c, *_ = np.linalg.lstsq(a, tgt, rcond=None)
    for _ in range(60):
        r = a @ c - tgt
        w = (np.abs(r) + 1e-12) ** 0.5
        c, *_ = np.linalg.lstsq(a * w[:, None], tgt * w, rcond=None)
    return c


_SINC = _fit_odd(np.sin, np.pi, 6)      # sin on [-pi, pi], deg-11 odd
_TANHC = _fit_odd(np.tanh, 1.0, 6)      # tanh on [-1, 1], deg-11 odd

_CACHE = {}


def _register_custom_ops():
    """Register fused DVE ops (idempotent). Shas are pinned by bootstrap:
    compile once with an empty pin, parse the actual sha from the error."""
    import re
    import concourse.dve_ops as dve_ops_mod
    from concourse.dve_ops import OPS, DveOp
    from concourse.dve_spec import Spec, Src0, Src1, C0, C1, C2, Zero

    have = {o.name for o in OPS}

    def make(name, spec):
        if name in have:
            return next(o for o in OPS if o.name == name)
        probe = DveOp(name, spec, subdim=False, uops_sha={})
        OPS.append(probe)
        dve_ops_mod._SUB_OPCODE_FOR_NAME[name] = (
            dve_ops_mod._CUSTOM_DVE_ROW_BASE + len(OPS) - 1)
        shas = {}
        for ver in ("v3", "v4"):
            try:
                probe.compile(ver)
            except ValueError as e:
                mm = re.search(r"(\b[0-9a-f]{16})\b", str(e))
                shas[ver] = mm.group(1)
        OPS.remove(probe)
        op = DveOp(name, spec, subdim=False, uops_sha=shas)
        OPS.append(op)
        return op

    # out = wrap(in0 + in1 + c0) into [-c1, c1] by one period imm2
    y = (Src0 + Src1) + C0
    addwrap2 = make("ADDWRAP2_ANT", Spec(
        body=y + C2 * ((y < (Zero - C1)) - (C1 < y)),
        reference=lambda in0, in1, c0, c1, c2: (
            lambda yy: (yy + c2 * ((yy < -c1).astype(np.float32)
                                   - (c1 < yy).astype(np.float32))
                        ).astype(np.float32))(
            (in0.astype(np.float32) + in1 + c0).astype(np.float32))))
    # out = ((c0*y + c1)*y + c2)*y, y = in0^2   (first half of odd Horner)
    y0 = Src0 * Src0
    oddpa = make("ODDPA_ANT", Spec(
        body=((C0 * y0 + C1) * y0 + C2) * y0,
        reference=lambda in0, in1, c0, c1, c2: (
            lambda yy: (((c0 * yy + c1) * yy + c2) * yy).astype(np.float32))(
            (in0.astype(np.float32) ** 2))))
    # out = ((((in0 + c0)*y) + c1)*y + c2)*in1, y = in1^2  (second half)
    y1 = Src1 * Src1
    oddpb = make("ODDPB_ANT", Spec(
        body=((((Src0 + C0) * y1) + C1) * y1 + C2) * Src1,
        reference=lambda in0, in1, c0, c1, c2: (
            lambda yy: (((((in0 + c0) * yy) + c1) * yy + c2) * in1
                        ).astype(np.float32))(
            (in1.astype(np.float32) ** 2))))
    # out = cumsum(in0 * in1) along the free stream
    from concourse.dve_spec import scan, AluOp

    def _ref_mulscan(in0, in1, c0, c1, c2):
        p = (in0.astype(np.float32) * in1).reshape(in0.shape[0], -1)
        return np.cumsum(p, axis=1, dtype=np.float32).reshape(in0.shape)

    mulscan = make("MULSCAN_ANT", Spec(
        body=scan(AluOp.ADD, Src0 * Src1),
        reference=_ref_mulscan))
    return addwrap2, oddpa, oddpb, mulscan


def _build_program(debug=False):
    """Build + compile the (weights-independent) single-core SPMD Bass program."""
    import concourse.bass as bass
    import concourse.mybir as mybir
    import concourse.tile as tile
    from concourse import bacc

    F32 = mybir.dt.float32
    AF = mybir.ActivationFunctionType
    OP = mybir.AluOpType

    CUSTOM_OPS = _register_custom_ops()

    nc = bacc.Bacc(None, target_bir_lowering=False)

    x_d = nc.dram_tensor("xs", [BPC, SEQ * 4], F32, kind="ExternalInput")
    wh_d = nc.dram_tensor("wh", [128, 128], F32, kind="ExternalInput")
    cp_d = nc.dram_tensor("cp", [128, 37], F32, kind="ExternalInput")
    y_d = nc.dram_tensor("y", [BPC, SEQ], F32, kind="ExternalOutput")
    if debug:
        dv_d = nc.dram_tensor("dbg_v", [128, GRP * SEQ], F32, kind="ExternalOutput")
        dp_d = nc.dram_tensor("dbg_p", [128, SEQ * GRP * 16], F32, kind="ExternalOutput")
        dh_d = nc.dram_tensor("dbg_h", [128, (SEQ + 1) * GRP * 4], F32, kind="ExternalOutput")

    with tile.TileContext(nc) as tc:
        with (
            tc.tile_pool(name="big", bufs=1) as big,
            tc.tile_pool(name="ph1", bufs=1) as ph1,
            tc.tile_pool(name="step", bufs=2) as sp,
        ):
            # ---------------- load ----------------
            xsb = big.tile([128, GRP * SEQ * 4], F32, tag="X")       # (g, t, k)
            nc.sync.dma_start(
                xsb[:].rearrange("p (g n) -> p g n", g=GRP),
                x_d.rearrange("(g p) n -> p g n", p=128),
            )
            whsb = big.tile([128, 128], F32, tag="WH")               # (g, a, w, k)
            nc.sync.dma_start(whsb[:], wh_d[:])
            cpsb = big.tile([128, 37], F32, tag="CP")
            nc.sync.dma_start(cpsb[:], cp_d[:])

            # ---------------- phase 1: bits -> conv ----------------
            bsb = big.tile([128, GRP * SEQ * 4], F32, tag="B")
            nc.vector.tensor_scalar(out=bsb[:], in0=xsb[:], scalar1=127.0,
                                    scalar2=None, op0=OP.is_gt)
            bv = bsb[:].rearrange("p (g t k) -> p g t k", g=GRP, k=4)
            bk = [bv[:, :, :, k] for k in range(4)]                  # each (128, g, t)

            q01 = ph1.tile([128, GRP * SEQ], F32, tag="q01")
            q23 = ph1.tile([128, GRP * SEQ], F32, tag="q23")
            gt = lambda tl: tl[:].rearrange("p (g t) -> p g t", g=GRP)
            nc.vector.tensor_tensor(out=gt(q01), in0=bk[0], in1=bk[1], op=OP.mult)
            nc.vector.tensor_tensor(out=gt(q23), in0=bk[2], in1=bk[3], op=OP.mult)
            rs = []
            for j in range(4):
                r = ph1.tile([128, GRP * SEQ], F32, tag=f"r{j}")
                nc.vector.tensor_scalar(out=gt(r), in0=bk[2],
                                        scalar1=float(_CML[j, 1]),
                                        scalar2=float(_CML[j, 0]),
                                        op0=OP.mult, op1=OP.add)
                nc.vector.affine_then_add(out=gt(r), in0=bk[3], in1=gt(r),
                                          scale=float(_CML[j, 2]), bias=0.0)
                nc.vector.affine_then_add(out=gt(r), in0=gt(q23), in1=gt(r),
                                          scale=float(_CML[j, 3]), bias=0.0)
                rs.append(r)
            m = ph1.tile([128, GRP * SEQ], F32, tag="m")
            vcv = big.tile([128, GRP * SEQ], F32, tag="V")           # conv (g, t)
            nc.vector.tensor_tensor(out=gt(m), in0=bk[0], in1=gt(rs[1]), op=OP.mult)
            nc.vector.tensor_tensor(out=gt(vcv), in0=gt(rs[0]), in1=gt(m), op=OP.add)
            nc.vector.tensor_tensor(out=gt(m), in0=bk[1], in1=gt(rs[2]), op=OP.mult)
            nc.vector.tensor_tensor(out=gt(vcv), in0=gt(vcv), in1=gt(m), op=OP.add)
            nc.vector.tensor_tensor(out=gt(m), in0=gt(q01), in1=gt(rs[3]), op=OP.mult)
            nc.vector.tensor_tensor(out=gt(vcv), in0=gt(vcv), in1=gt(m), op=OP.add)

            # ---------------- phase 1b: pre[t, g, a, w] = Wx*conv + beta ----
            pre = big.tile([128, SEQ * GRP * 16], F32, tag="P")
            pv = pre[:].rearrange("p (t g a w) -> p t g a w", t=SEQ, g=GRP, a=4)
            vt = vcv[:].rearrange("p (g t) -> p g t", g=GRP).transpose([0, 2, 1])
            for a in range(4):
                for w in range(4):
                    j = a * 4 + w
                    nc.vector.tensor_scalar(
                        out=pv[:, :, :, a, w], in0=vt,
                        scalar1=cpsb[:, j:j + 1],
                        scalar2=cpsb[:, 16 + j:17 + j],
                        op0=OP.mult, op1=OP.add)

            # ---------------- phase 2: the scan ----------------
            # Joint chain over both batch groups; sin/tanh as 2-op DVE odd
            # polynomials (only tanh(c) stays on ACT); side products on
            # gpsimd to overlap with the DVE critical path.
            addwrap2, oddpa, oddpb, mulscan = CUSTOM_OPS
            sc = [float(v) for v in _SINC]
            tc = [float(v) for v in _TANHC]
            hs = big.tile([128, (SEQ + 1) * GRP * 4], F32, tag="HS")  # 2h, (t, g, w)
            s2 = big.tile([128, GRP * 4], F32, tag="S2")              # 2c
            # prefix-sum scratch: per group a zero guard col + 64 scan cols
            sg = big.tile([128, GRP * 65], F32, tag="SG")
            nc.vector.memset(hs[:, 0:GRP * 4], 0.0)
            nc.vector.memset(s2[:], 0.0)
            nc.vector.memset(sg[:], 0.0)
            side = nc.gpsimd if SIDE_ON_GPSIMD else nc.vector

            for t in range(SEQ):
                # theta_h[m] = sum_k wh[m,k]*h[k] via per-group fused
                # multiply-cumsum, then stride-4 difference of page ends.
                for g in range(GRP):
                    hprev = hs[:, (t * GRP + g) * 4:(t * GRP + g + 1) * 4]
                    hb = hprev.unsqueeze(1).broadcast_to((128, 16, 4))
                    nc.vector._custom_dve(
                        mulscan,
                        out=sg[:, g * 65 + 1:g * 65 + 65]
                            .rearrange("p (r k) -> p r k", k=4),
                        in0=hb,
                        in1=whsb[:, g * 64:(g + 1) * 64]
                            .rearrange("p (r k) -> p r k", k=4))
                th = sp.tile([128, GRP * 16], F32, tag="th")
                sgv = sg[:].rearrange("p (g c) -> p g c", g=GRP)
                nc.vector.tensor_tensor(
                    out=th[:].rearrange("p (g m) -> p g m", g=GRP),
                    in0=sgv[:, :, 1:65].rearrange("p g (m k) -> p g m k", k=4)[:, :, :, 3],
                    in1=sgv[:, :, 0:64].rearrange("p g (m k) -> p g m k", k=4)[:, :, :, 0],
                    op=OP.subtract)
                # wr = wrap(th + pre + pi/2) into [-pi, pi]
                wr = sp.tile([128, GRP * 16], F32, tag="wr")
                nc.vector._custom_dve(
                    addwrap2, out=wr[:], in0=th[:],
                    in1=pre[:, t * GRP * 16:(t + 1) * GRP * 16],
                    s0=PI / 2, s1=PI, imm2=2 * PI)
                # z = sin(wr) via deg-11 odd polynomial (2 fused DVE ops)
                sa = sp.tile([128, GRP * 16], F32, tag="sa")
                nc.vector._custom_dve(oddpa, out=sa[:], in0=wr[:],
                                      s0=sc[5], s1=sc[4], imm2=sc[3])
                zz = sp.tile([128, GRP * 16], F32, tag="zz")
                nc.vector._custom_dve(oddpb, out=zz[:], in0=sa[:], in1=wr[:],
                                      s0=sc[2], s1=sc[1], imm2=sc[0])
                z4 = zz[:].rearrange("p (g a w) -> p g a w", g=GRP, a=4)
                qq = sp.tile([128, GRP * 16], F32, tag="qq")
                q4 = qq[:].rearrange("p (g a w) -> p g a w", g=GRP, a=4)
                b23 = sp.tile([128, GRP * 4], F32, tag="b23")
                b23v = b23[:].rearrange("p (g a) -> p g a", g=GRP)
                # q1 = z0*z1 (x0.5 for sigmoid gates f,i,o)
                nc.vector.scalar_tensor_tensor(
                    out=q4[:, :, 0:3, 1], in0=z4[:, :, 0:3, 1], scalar=0.5,
                    in1=z4[:, :, 0:3, 0], op0=OP.mult, op1=OP.mult)
                side.tensor_tensor(out=q4[:, :, 3, 1], in0=z4[:, :, 3, 1],
                                   in1=z4[:, :, 3, 0], op=OP.mult)
                side.tensor_tensor(out=b23v, in0=z4[:, :, :, 2],
                                   in1=z4[:, :, :, 3], op=OP.mult)
                # q0 = z1'*b23
                side.scalar_tensor_tensor(
                    out=q4[:, :, 0:3, 0], in0=z4[:, :, 0:3, 1], scalar=0.5,
                    in1=b23v[:, :, 0:3], op0=OP.mult, op1=OP.mult)
                side.tensor_tensor(out=q4[:, :, 3, 0], in0=z4[:, :, 3, 1],
                                   in1=b23v[:, :, 3], op=OP.mult)
                # q2 = q1*z2 ; q3 = q1*b23
                nc.vector.tensor_tensor(out=q4[:, :, :, 2], in0=q4[:, :, :, 1],
                                        in1=z4[:, :, :, 2], op=OP.mult)
                side.tensor_tensor(out=q4[:, :, :, 3], in0=q4[:, :, :, 1],
                                   in1=b23v, op=OP.mult)
                # T = tanh(q) via deg-11 odd polynomial (2 fused DVE ops)
                ta = sp.tile([128, GRP * 16], F32, tag="ta")
                nc.vector._custom_dve(oddpa, out=ta[:], in0=qq[:],
                                      s0=tc[5], s1=tc[4], imm2=tc[3])
                tt_ = sp.tile([128, GRP * 16], F32, tag="tt")
                nc.vector._custom_dve(oddpb, out=tt_[:], in0=ta[:], in1=qq[:],
                                      s0=tc[2], s1=tc[1], imm2=tc[0])
                t4 = tt_[:].rearrange("p (g a w) -> p g a w", g=GRP, a=4)
                s1 = sp.tile([128, GRP * 4], F32, tag="s1")
                nc.vector.scalar_tensor_tensor(
                    out=s1[:].rearrange("p (g w) -> p g w", g=GRP),
                    in0=t4[:, :, 0, :], scalar=1.0,
                    in1=s2[:].rearrange("p (g w) -> p g w", g=GRP),
                    op0=OP.add, op1=OP.mult)
                s2t = sp.tile([128, GRP * 4], F32, tag="s2t")
                side.scalar_tensor_tensor(
                    out=s2t[:].rearrange("p (g w) -> p g w", g=GRP),
                    in0=t4[:, :, 1, :], scalar=1.0, in1=t4[:, :, 3, :],
                    op0=OP.add, op1=OP.mult)
                nc.vector.scalar_tensor_tensor(
                    out=s2[:], in0=s1[:], scalar=0.5, in1=s2t[:],
                    op0=OP.mult, op1=OP.add)
                tcn = sp.tile([128, GRP * 4], F32, tag="tcn")
                nc.scalar.activation(tcn[:], s2[:], AF.Tanh, scale=0.5)
                hnew = hs[:, (t + 1) * GRP * 4:(t + 2) * GRP * 4]
                nc.vector.scalar_tensor_tensor(
                    out=hnew.rearrange("p (g w) -> p g w", g=GRP),
                    in0=t4[:, :, 2, :], scalar=1.0,
                    in1=tcn[:].rearrange("p (g w) -> p g w", g=GRP),
                    op0=OP.add, op1=OP.mult)

            # ---------------- phase 3: y = hs @ (W_out/2) + b_out ----------
            yt = big.tile([128, GRP * SEQ * 4], F32, tag="YT")
            hsv = (hs[:, GRP * 4:].rearrange("p (t g w) -> p t g w", t=SEQ, g=GRP)
                   .transpose([0, 2, 1, 3]))
            wo = (cpsb[:, 32:36].unsqueeze(1).unsqueeze(1)
                  .broadcast_to((128, GRP, SEQ, 4)))
            nc.vector.tensor_tensor(
                out=yt[:].rearrange("p (g t w) -> p g t w", g=GRP, t=SEQ),
                in0=hsv, in1=wo, op=OP.mult)
            yy = big.tile([128, GRP * SEQ], F32, tag="Y")
            nc.vector.tensor_reduce(
                out=yy[:], in_=yt[:].rearrange("p (m w) -> p m w", w=4),
                axis=mybir.AxisListType.X, op=OP.add)
            nc.vector.tensor_scalar(out=yy[:], in0=yy[:],
                                    scalar1=cpsb[:, 36:37], scalar2=None,
                                    op0=OP.add)
            nc.sync.dma_start(
                y_d.rearrange("(g p) t -> p g t", p=128),
                yy[:].rearrange("p (g t) -> p g t", g=GRP),
            )
            if debug:
                nc.sync.dma_start(dv_d[:], vcv[:])
                nc.sync.dma_start(dp_d[:], pre[:])
                nc.sync.dma_start(dh_d[:], hs[:])

    # Force a single ACT table set covering both Sin and Tanh; otherwise the
    # table-load pass alternates trig/tanh sets every scan step (~2.7us each).
    # Set ids are positional into act_info.json, so keep the full dict in
    # order and strip Sin/Tanh from every set except silu_and_others.
    import concourse.bacc as bacc_mod
    orig_gat = bacc_mod.get_activation_tables

    def filtered_gat(arch):
        tabs = {}
        for name, fns in orig_gat(arch).items():
            if name != "silu_and_others":
                fns = {f for f in fns
                       if f not in (AF.Sin, AF.Tanh)}
            tabs[name] = fns
        return tabs

    bacc_mod.get_activation_tables = filtered_gat
    try:
        nc.compile()
    finally:
        bacc_mod.get_activation_tables = orig_gat
    return nc


def _pack_consts(W_f, b_f, W_i, b_i, W_u, b_u, W_o, b_o,
                 rx_f, rx_i, rx_u, rx_o, W_out, b_out):
    """wh[128,128] and cp[128,37] constant tiles (replicated rows)."""
    Ws = [W_f, W_i, W_o, W_u]          # gate order f,i,o,u
    bs = [b_f, b_i, b_o, b_u]
    rxs = [rx_f, rx_i, rx_o, rx_u]
    whrow = np.zeros((GRP, 4, 4, 4), np.float32)
    for a in range(4):
        whrow[:, a, :, :] = 0.5 * np.asarray(Ws[a], np.float32)[:, 1:5][None]
    wh = np.tile(whrow.reshape(1, 128), (128, 1)).astype(np.float32)

    cprow = np.zeros(37, np.float32)
    for a in range(4):
        Wa = np.asarray(Ws[a], np.float32)
        cprow[a * 4:(a + 1) * 4] = Wa[:, 0]
        cprow[16 + a * 4:16 + (a + 1) * 4] = (
            np.asarray(bs[a], np.float32) + np.asarray(rxs[a], np.float32))
    cprow[32:36] = 0.5 * np.asarray(W_out, np.float32)[0]
    cprow[36] = float(np.asarray(b_out, np.float32)[0])
    cp = np.tile(cprow[None], (128, 1)).astype(np.float32)

    # range check for the single add_range_wrap before Sin
    wx = np.abs(cprow[0:16])
    beta = np.abs(cprow[16:32])
    whabs = np.abs(whrow[0]).reshape(16, 4).sum(axis=1) * 2.0  # |Wh| row sums
    bound = (wx + beta + whabs).max() + PI / 2
    assert bound < 3 * PI - 0.2, f"theta range {bound} too large for single wrap"
    return wh, cp


def kernel(**inputs):
    from concourse.bass_utils import run_bass_kernel_spmd

    x = np.ascontiguousarray(np.asarray(inputs["x"], np.float32)).reshape(BATCH, SEQ, 4)
    wh, cp = _pack_consts(**{k: v for k, v in inputs.items() if k != "x"})

    if "nc" not in _CACHE:
        _CACHE["nc"] = _build_program()
    nc = _CACHE["nc"]

    in_maps = []
    for cid in range(N_CORES):
        xs = np.ascontiguousarray(
            x[cid * BPC:(cid + 1) * BPC].reshape(BPC, SEQ * 4))
        in_maps.append({"xs": xs, "wh": wh, "cp": cp})

    res = run_bass_kernel_spmd(nc, in_maps, core_ids=list(range(N_CORES)))
    ys = [res.results[cid]["y"] for cid in range(N_CORES)]  # each (BPC, SEQ)
    full = np.concatenate(ys, axis=0)                       # (BATCH, SEQ)
    return np.ascontiguousarray(full.T)[:, :, None].astype(np.float32)

